# revision 17
# baseline (speedup 1.0000x reference)
"""GCN aggregator kernel for 8 Trainium2 NeuronCores (Bass/Tile), v2.

Computes: out = D_r^{-1/2} M D_c^{-1/2} E[unique_ids]  where M is the
[B, U] 0/1 neighbor mask built from neigh_cols (duplicate (row, col)
pairs collapse to 1).

v2 layout ("compute at u-home, reduce outputs"): instead of AllGathering
the full scaled table E' (16.8 MB collective, the v1 bottleneck), each
core keeps only its contiguous 4096-row u-shard of E' and processes ALL
B*K pairs against that shard (invalid pairs masked to 0); the per-core
partial outputs [B, D] are then ReduceScattered (256 KB out per core).
Collectives: AllGather of row-weights w (512 KB), ReduceScatter of the
column-count histogram (16 KB out), ReduceScatter of outputs.

Sparse decomposition (exact):
  f[b,k]   = 1 if k is the first position in row b with value neigh_cols[b,k]
  row_cnt  = sum_k f[b,k]            (distinct neighbors per row)
  col_cnt  = scatter-add of f by u   (rows containing u; global over B)
  out[b]   = sum_k f[b,k]*rsqrt(row_cnt[b]) * icn[u] * E[unique_ids[u]],
             u = neigh_cols[b,k],  icn[u] = rsqrt(max(col_cnt[u], 1))

Per-core layouts (core c):
  row b = 32p + t'  (p = partition, t' = 0..31); own w-rows: t' in [4c,4c+4)
  u global = 4096c + u_loc; E'-shard row u_loc = 32p + r at sbuf [p, r]
  histogram u = 128*hi + lo, psum [lo, hi]; hist-own rows t' in [4c,4c+4)
  pair i = 16384j + 128*(32*tl + k) + p, t' = 4j + tl; gathered rows are
  E'_c fp16, weighted by w[b,k]*valid and k-reduced via a strided view.
"""

import os
import numpy as np
from contextlib import ExitStack

import concourse.tile as tile
from concourse import bass, bacc, mybir
from concourse.bass_utils import run_bass_kernel_spmd

dt = mybir.dt
Alu = mybir.AluOpType
Act = mybir.ActivationFunctionType

B, K, U, V, D = 4096, 32, 32768, 100000, 128
NC = 8
BC = B // NC            # 512 output rows per core
USH = U // NC           # 4096 unique ids per core (u-shard)
R = USH // 128          # 32 shard rows per partition
TL = 4                  # own t'-slice width (32 / NC)
NPAIR = B * K           # 131072 pairs, processed on every core
NCH = 16                # pair chunks
CP = NPAIR // NCH       # 8192 pairs per chunk
GPC = CP // 128         # 64 gather groups per chunk
TCH = 32 // NCH         # t'-slices per chunk

LAST_RESULTS = None     # test harness reads profiling info from here
_PROGRAM = None


def _build_program():
    skips = set(os.environ.get("GCN_SKIP", "").split(","))
    nc = bacc.Bacc("TRN2", target_bir_lowering=False, debug=False, num_devices=NC)

    t_xo = nc.dram_tensor("xo", [128, TL * K], dt.int32, kind="ExternalInput").ap()
    t_valid = nc.dram_tensor("valid", [128, 32 * K], dt.float32,
                             kind="ExternalInput").ap()
    t_ids = nc.dram_tensor("ids", [128, R], dt.int32, kind="ExternalInput").ap()
    t_emb = nc.dram_tensor("emb", [V, D], dt.float32, kind="ExternalInput").ap()
    t_iota = nc.dram_tensor("iotaf", [128, 256], dt.float32,
                            kind="ExternalInput").ap()
    t_c127 = nc.dram_tensor("c127", [128, TL * K], dt.int32,
                            kind="ExternalInput").ap()
    t_idxw = nc.dram_tensor("idxw", [128, NPAIR // 16], dt.int16,
                            kind="ExternalInput").ap()
    t_id128 = nc.dram_tensor("id128", [128, 128], dt.float32,
                             kind="ExternalInput").ap()
    t_out = nc.dram_tensor("out", [BC, D], dt.float32, kind="ExternalOutput").ap()

    # standalone DRAM scratch (offset-0 APs for collectives / indirect reads)
    t_cnt_in = nc.dram_tensor("cnt_in", [256, 128], dt.float32).ap()
    t_cnt_rs = nc.dram_tensor("cnt_rs", [32, 128], dt.float32).ap()
    t_icnf = nc.dram_tensor("icnf", [USH], dt.float32).ap()
    t_ec = nc.dram_tensor("ec", [USH, D], dt.float16).ap()
    t_w_in = nc.dram_tensor("w_in", [BC * K], dt.float32).ap()
    t_w_full = nc.dram_tensor("w_full", [NPAIR], dt.float32,
                              addr_space="Shared").ap()
    t_po = nc.dram_tensor("po", [B, D], dt.float32).ap()
    t_rso = nc.dram_tensor("rso", [BC, D], dt.float32).ap()

    with tile.TileContext(nc) as tc, ExitStack() as ctx:
        sb = ctx.enter_context(tc.tile_pool(name="sb", bufs=1))
        gpool = ctx.enter_context(tc.tile_pool(name="gp", bufs=4))
        tp = ctx.enter_context(tc.tile_pool(name="tp", bufs=2))
        ps = ctx.enter_context(tc.tile_pool(name="ps", bufs=1, space="PSUM"))
        psd = ctx.enter_context(tc.tile_pool(name="psd", bufs=2, space="PSUM"))

        # ---------- loads ----------
        s_ids = sb.tile([128, R], dt.int32)
        nc.sync.dma_start(s_ids[:], t_ids)
        s_xo = sb.tile([128, TL, K], dt.int32)
        nc.sync.dma_start(s_xo[:], t_xo.rearrange("p (t k) -> p t k", t=TL))
        s_iota = sb.tile([128, 256], dt.float32)
        nc.sync.dma_start(s_iota[:], t_iota)
        s_c127 = sb.tile([128, TL, K], dt.int32)
        nc.sync.dma_start(s_c127[:], t_c127.rearrange("p (t k) -> p t k", t=TL))
        s_id128 = sb.tile([128, 128], dt.float32)
        nc.sync.dma_start(s_id128[:], t_id128)
        s_idxw = sb.tile([128, NPAIR // 16], dt.int16)
        nc.sync.dma_start(s_idxw[:], t_idxw)
        s_valid = sb.tile([128, 32, K], dt.float32)
        nc.sync.dma_start(s_valid[:], t_valid.rearrange("p (t k) -> p t k", t=32))

        # ---------- E'-shard raw gather (early; Pool engine) ----------
        s_eraw = sb.tile([128, R, D], dt.float32)
        for r in range(1 if "ebuild" in skips else R):
            nc.gpsimd.indirect_dma_start(
                out=s_eraw[:, r, :], out_offset=None, in_=t_emb,
                in_offset=bass.IndirectOffsetOnAxis(ap=s_ids[:, r:r + 1], axis=0))

        # ---------- first-occurrence mask + row weights (own t'-slice) ----------
        s_xf = sb.tile([128, TL, K], dt.float32)
        nc.vector.tensor_copy(s_xf[:], s_xo[:])
        s_dup = sb.tile([128, TL, K], dt.float32)
        nc.vector.memset(s_dup[:], 0.0)
        s_eq = sb.tile([128, TL, K], dt.float32)
        for sh in range(1, (2 if "focc" in skips else K)):
            w = K - sh
            nc.vector.tensor_tensor(
                out=s_eq[:, :, :w], in0=s_xf[:, :, sh:K], in1=s_xf[:, :, 0:w],
                op=Alu.is_equal)
            nc.vector.tensor_tensor(
                out=s_dup[:, :, sh:K], in0=s_dup[:, :, sh:K], in1=s_eq[:, :, :w],
                op=Alu.add)
        s_f = sb.tile([128, TL, K], dt.float32)
        nc.vector.tensor_scalar(
            out=s_f[:], in0=s_dup[:], scalar1=0.0, scalar2=None, op0=Alu.is_equal)
        s_rcnt = sb.tile([128, TL], dt.float32)
        nc.vector.tensor_reduce(
            out=s_rcnt[:], in_=s_f[:], axis=mybir.AxisListType.X, op=Alu.add)
        s_rsq = sb.tile([128, TL], dt.float32)
        nc.scalar.activation(out=s_rsq[:], in_=s_rcnt[:], func=Act.Sqrt)
        s_rn = sb.tile([128, TL], dt.float32)
        nc.vector.reciprocal(out=s_rn[:], in_=s_rsq[:])
        s_s = sb.tile([128, TL, K], dt.float32)
        nc.vector.tensor_tensor(
            out=s_s[:], in0=s_f[:], in1=s_rn[:].to_broadcast([128, TL, K]),
            op=Alu.mult)


        # ---------- histogram of own t'-slice: psum[lo, hi] ----------
        s_lo = sb.tile([128, TL, K], dt.int32)
        nc.vector.tensor_tensor(
            out=s_lo[:], in0=s_xo[:], in1=s_c127[:], op=Alu.bitwise_and)
        s_lof = sb.tile([128, TL, K], dt.float32)
        nc.vector.tensor_copy(s_lof[:], s_lo[:])
        s_hif = sb.tile([128, TL, K], dt.float32)
        nc.vector.tensor_tensor(
            out=s_hif[:], in0=s_xf[:], in1=s_lof[:], op=Alu.subtract)
        s_hifs = sb.tile([128, TL, K], dt.float32)
        nc.vector.tensor_scalar(
            out=s_hifs[:], in0=s_hif[:], scalar1=1.0 / 128.0, scalar2=None,
            op0=Alu.mult)

        p_hist = ps.tile([128, 256], dt.float32, space="PSUM")
        s_iota16 = sb.tile([128, 256], dt.bfloat16)
        nc.vector.tensor_copy(s_iota16[:], s_iota[:])
        ntl = 1 if "hist" in skips else TL
        for tl in range(ntl):
            lo_oh = sb.tile([128, K, 128], dt.bfloat16, tag="looh")
            hi_oh = sb.tile([128, K, 256], dt.bfloat16, tag="hioh")
            for k in range(K):
                nc.vector.tensor_scalar(
                    out=lo_oh[:, k, :], in0=s_iota16[:, 0:128],
                    scalar1=s_lof[:, tl, k:k + 1], scalar2=None,
                    op0=Alu.is_equal)
                nc.vector.tensor_scalar(
                    out=hi_oh[:, k, :], in0=s_iota16[:, 0:256],
                    scalar1=s_hifs[:, tl, k:k + 1], scalar2=s_f[:, tl, k:k + 1],
                    op0=Alu.is_equal, op1=Alu.mult)
            for k in range(K):
                nc.tensor.matmul(
                    p_hist[:], lhsT=lo_oh[:, k, :], rhs=hi_oh[:, k, :],
                    start=(tl == 0 and k == 0),
                    stop=(tl == ntl - 1 and k == K - 1))
        s_hist = sb.tile([128, 256], dt.float32)
        nc.vector.tensor_copy(s_hist[:], p_hist[:])

        # transpose to [hi, lo] u-major layout for the ReduceScatter
        s_tr2 = sb.tile([128, 2, 128], dt.float32)
        for h in range(2):
            p_tr = psd.tile([128, 128], dt.float32, space="PSUM", tag="ptr")
            nc.tensor.transpose(
                out=p_tr[:], in_=s_hist[:, 128 * h:128 * (h + 1)],
                identity=s_id128[:])
            nc.vector.tensor_copy(s_tr2[:, h, :], p_tr[:])
        nc.sync.dma_start(
            t_cnt_in.rearrange("(h p) c -> p h c", h=2), s_tr2[:])
        if "coll" in skips:
            nc.gpsimd.dma_start(t_cnt_rs[:], t_cnt_in[0:32, :])
        else:
            nc.gpsimd.collective_compute(
                "ReduceScatter", Alu.add, replica_groups=[list(range(NC))],
                ins=[t_cnt_in], outs=[t_cnt_rs])

        # ---------- AllGather of row weights w = f * rsqrt(row_cnt) ----------
        # (issued after the RS so the cnt result is not queued behind it)
        nc.sync.dma_start(
            t_w_in.rearrange("(p q) -> p q", p=128),
            s_s[:].rearrange("p t k -> p (t k)"))
        if "coll" in skips:
            nc.gpsimd.dma_start(t_w_full[0:BC * K], t_w_in[:])
        else:
            nc.gpsimd.collective_compute(
                "AllGather", Alu.bypass, replica_groups=[list(range(NC))],
                ins=[t_w_in], outs=[t_w_full])
        s_w = sb.tile([128, 32, K], dt.float32)
        nc.sync.dma_start(
            s_w[:].rearrange("p (c t) k -> p c t k", c=NC),
            t_w_full.rearrange("(c p t k) -> p c t k", c=NC, p=128, t=TL))
        nc.vector.tensor_tensor(
            out=s_w[:], in0=s_w[:], in1=s_valid[:], op=Alu.mult)
        s_w16 = sb.tile([128, 32, K], dt.float16)
        nc.vector.tensor_copy(s_w16[:], s_w[:])

        # ---------- icn = rsqrt(max(cnt, 1)) for own shard; relayout ----------
        s_csh = sb.tile([32, 128], dt.float32)
        nc.sync.dma_start(s_csh[:], t_cnt_rs)
        s_icsq = sb.tile([32, 128], dt.float32)
        nc.vector.tensor_scalar(
            out=s_icsq[:], in0=s_csh[:], scalar1=1.0, scalar2=None, op0=Alu.max)
        nc.scalar.activation(out=s_icsq[:], in_=s_icsq[:], func=Act.Sqrt)
        s_icn = sb.tile([32, 128], dt.float32)
        nc.vector.reciprocal(out=s_icn[:], in_=s_icsq[:])
        nc.sync.dma_start(t_icnf.rearrange("(h l) -> h l", h=32), s_icn[:])
        s_icnpr = sb.tile([128, R], dt.float32)
        nc.sync.dma_start(s_icnpr[:], t_icnf.rearrange("(p r) -> p r", p=128))

        # ---------- scale shard to fp16 and stage in DRAM ----------
        s_ec = sb.tile([128, R, D], dt.float16)
        nc.vector.tensor_tensor(
            out=s_ec[:], in0=s_eraw[:],
            in1=s_icnpr[:].to_broadcast([128, R, D]), op=Alu.mult)
        nc.sync.dma_start(t_ec.rearrange("(p r) d -> p r d", p=128), s_ec[:])

        # ---------- main gather + masked weighted k-reduction ----------
        s_acc = sb.tile([128, 32, D], dt.float32)
        if "wsum" in skips or "gather" in skips:
            nc.vector.memset(s_acc[:], 0.0)
        for j in range(0 if "gather" in skips else NCH):
            s_g = gpool.tile([128, GPC, D], dt.float16, tag="gch")
            nc.gpsimd.dma_gather(
                out_ap=s_g[:], in_ap=t_ec,
                idxs_ap=s_idxw[:, (CP // 16) * j:(CP // 16) * (j + 1)],
                num_idxs=CP, num_idxs_reg=CP, elem_size=D,
                single_packet=False)
            if "wsum" in skips:
                continue
            gv = s_g[:].rearrange("p (t k) d -> p t k d", t=TCH)
            nc.vector.tensor_tensor(
                out=gv, in0=gv,
                in1=s_w16[:, TCH * j:TCH * (j + 1), :]
                    .to_broadcast([128, TCH, K, D]),
                op=Alu.mult)
            # packed fp16 pairwise-add tree over k (2x DVE), final level to f32
            s_h = tp.tile([128, TCH, K // 2, D], dt.float16, tag="htree")
            nc.vector.tensor_tensor(
                out=s_h[:], in0=gv[:, :, 0:K // 2, :], in1=gv[:, :, K // 2:K, :],
                op=Alu.add)
            m = K // 4
            while m >= 2:
                nc.vector.tensor_tensor(
                    out=s_h[:, :, 0:m, :], in0=s_h[:, :, 0:m, :],
                    in1=s_h[:, :, m:2 * m, :], op=Alu.add)
                m //= 2
            nc.vector.tensor_tensor(
                out=s_acc[:, TCH * j:TCH * (j + 1), :],
                in0=s_h[:, :, 0, :], in1=s_h[:, :, 1, :], op=Alu.add)

        nc.sync.dma_start(t_po.rearrange("(p t) d -> p t d", t=32), s_acc[:])
        if "coll" in skips:
            nc.gpsimd.dma_start(t_rso[:], t_po[0:BC, :])
        else:
            nc.gpsimd.collective_compute(
                "ReduceScatter", Alu.add, replica_groups=[list(range(NC))],
                ins=[t_po], outs=[t_rso])
        s_out = sb.tile([128, TL, D], dt.float32)
        nc.sync.dma_start(s_out[:], t_rso.rearrange("(p t) d -> p t d", t=TL))
        nc.sync.dma_start(t_out.rearrange("(p t) d -> p t d", t=TL), s_out[:])

    nc.compile()
    return nc


def _get_program():
    global _PROGRAM
    if _PROGRAM is None:
        _PROGRAM = _build_program()
    return _PROGRAM


def _make_in_maps(neigh_cols, unique_ids, embed_table):
    x = np.ascontiguousarray(np.asarray(neigh_cols, dtype=np.int32))
    uids = np.ascontiguousarray(np.asarray(unique_ids, dtype=np.int32))
    emb = np.ascontiguousarray(np.asarray(embed_table, dtype=np.float32))
    iotaf = np.broadcast_to(np.arange(256, dtype=np.float32), (128, 256)).copy()
    c127 = np.full((128, TL * K), 127, np.int32)
    id128 = np.eye(128, dtype=np.float32)

    # pair order: i = 16384*j + 128*(32*tl + k) + p ; t' = 4j + tl ; b = 32p + t'
    i = np.arange(NPAIR)
    j, rem = np.divmod(i, CP)
    g, p = np.divmod(rem, 128)
    tl, k = np.divmod(g, K)
    tprime = TCH * j + tl
    b = 32 * p + tprime
    ub = x[b, k]                      # u of pair i

    # valid[p, t', k] and x_own are per-core; b for [p, t', k] layout:
    pp = np.arange(128)[:, None, None]
    tt = np.arange(32)[None, :, None]
    kk = np.arange(K)[None, None, :]
    x_ptk = x[32 * pp + tt, kk]       # [128, 32, 32]

    in_maps = []
    for c in range(NC):
        vals = np.clip(ub - USH * c, 0, USH - 1).astype(np.int16)
        idxw = np.zeros((16, NPAIR // 16), np.int16)
        idxw[i % 16, i // 16] = vals
        idxw = np.tile(idxw, (8, 1))
        valid = (x_ptk // USH == c).astype(np.float32)
        x_own = x_ptk[:, TL * c:TL * (c + 1), :]       # t' in [4c, 4c+4)
        ids_c = uids[USH * c:USH * (c + 1)].reshape(128, R)
        in_maps.append({
            "xo": np.ascontiguousarray(x_own.reshape(128, TL * K)),
            "valid": np.ascontiguousarray(valid.reshape(128, 32 * K)),
            "ids": ids_c,
            "emb": emb,
            "iotaf": iotaf,
            "c127": c127,
            "idxw": idxw,
            "id128": id128,
        })
    return in_maps


def kernel(neigh_cols, unique_ids, embed_table):
    global LAST_RESULTS
    nc = _get_program()
    in_maps = _make_in_maps(neigh_cols, unique_ids, embed_table)
    trace = bool(int(os.environ.get("GCN_TRACE", "0")))
    res = run_bass_kernel_spmd(nc, in_maps, list(range(NC)), trace=trace)
    LAST_RESULTS = res
    out = np.concatenate([res.results[c]["out"] for c in range(NC)], axis=0)
    return out.astype(np.float32)


def bench_exec(inputs, iters=12):
    """Steady-state wall times (us) of the compiled NEFF via a reusable
    sharded jit with device-resident inputs. Excludes compile; includes
    per-call dispatch overhead of the runtime."""
    import time
    import jax
    from jax.sharding import Mesh, PartitionSpec, NamedSharding
    from jax.experimental.shard_map import shard_map
    from concourse.bass2jax import (_bass_exec_p, partition_id_tensor,
                                    install_neuronx_cc_hook)

    nc = _get_program()
    install_neuronx_cc_hook()
    in_maps = _make_in_maps(**inputs)

    partition_name = (nc.partition_id_tensor.name
                      if nc.partition_id_tensor else None)
    in_names, out_names, out_avals, zero_outs = [], [], [], []
    for alloc in nc.m.functions[0].allocations:
        if not isinstance(alloc, mybir.MemoryLocationSet):
            continue
        name = alloc.memorylocations[0].name
        if alloc.kind == "ExternalInput":
            if name != partition_name:
                in_names.append(name)
        elif alloc.kind == "ExternalOutput":
            out_names.append(name)
            shape = tuple(alloc.tensor_shape)
            npdt = dt.np(alloc.dtype)
            out_avals.append(jax.core.ShapedArray(shape, npdt))
            zero_outs.append(np.zeros(shape, npdt))
    n_params = len(in_names)
    all_names = in_names + out_names + ([partition_name] if partition_name else [])

    def _body(*args):
        operands = list(args)
        if partition_name is not None:
            operands.append(partition_id_tensor())
        return tuple(_bass_exec_p.bind(
            *operands, out_avals=tuple(out_avals), in_names=tuple(all_names),
            out_names=tuple(out_names), lowering_input_output_aliases=(),
            sim_require_finite=True, sim_require_nnan=True, nc=nc))

    devices = jax.devices()[:NC]
    mesh = Mesh(np.asarray(devices), ("core",))
    sharded = jax.jit(
        shard_map(_body, mesh=mesh,
                  in_specs=(PartitionSpec("core"),) * (n_params + len(out_names)),
                  out_specs=(PartitionSpec("core"),) * len(out_names),
                  check_rep=False),
        keep_unused=True)
    sh = NamedSharding(mesh, PartitionSpec("core"))
    concat_in = [jax.device_put(
        np.concatenate([np.asarray(in_maps[c][nm]) for c in range(NC)], axis=0),
        sh) for nm in in_names]
    concat_zero = [jax.device_put(
        np.zeros((NC * z.shape[0], *z.shape[1:]), z.dtype), sh)
        for z in zero_outs]
    out = sharded(*concat_in, *concat_zero)
    jax.block_until_ready(out)
    times = []
    for _ in range(iters):
        t0 = time.perf_counter()
        out = sharded(*concat_in, *concat_zero)
        jax.block_until_ready(out)
        times.append((time.perf_counter() - t0) * 1e6)
    return sorted(times)


def modeled_time_ns():
    """Single-core device-occupancy model of the program (cost-model sim)."""
    from concourse.timeline_sim import TimelineSim
    return TimelineSim(_get_program(), trace=False).simulate()


# revision 24
# speedup vs baseline: 1.1610x; 1.1610x over previous
"""GCN aggregator kernel for 8 Trainium2 NeuronCores (Bass/Tile), v3.

Computes: out = D_r^{-1/2} M D_c^{-1/2} E[unique_ids]  where M is the
[B, U] 0/1 neighbor mask built from neigh_cols (duplicate (row, col)
pairs collapse to 1).

v3 layout ("compute at u-home, 2-core table groups, reduce outputs"):
the v1 bottleneck was a 16.8 MB AllGather of the scaled table E'
(265 us of a 400 us kernel in the collective cost model). Instead:
  - each core builds its 4096-row u-shard of E' = icn * E[ids] locally
    and AllGathers it only within a 2-core group {c%4, c%4+4}
    (2 MB fp16 out, ~67 us), giving the group an 8192-row table;
  - each core processes HALF of B (rows [2048*(c//4), +2048)) x K pairs
    against the group table (out-of-group pairs masked to 0), so the
    descriptor-bound pair gather and the DVE weighted-sum halve vs. an
    all-pairs-per-core scheme;
  - row weights w = f * rsqrt(row_cnt) are computed for 512 own rows
    and AllGathered within [[0..3],[4..7]] (256 KB out);
  - the column-count histogram is ReduceScattered over all 8 cores;
  - partial outputs [2048, D] are ReduceScattered within
    [[0..3],[4..7]], landing each core exactly its [512, D] output.

Sparse decomposition (exact):
  f[b,k]   = 1 if k is the first position in row b with value neigh_cols[b,k]
  row_cnt  = sum_k f[b,k]            (distinct neighbors per row)
  col_cnt  = scatter-add of f by u   (rows containing u; global over B)
  out[b]   = sum_k f[b,k]*rsqrt(row_cnt[b]) * icn[u] * E[unique_ids[u]],
             u = neigh_cols[b,k],  icn[u] = rsqrt(max(col_cnt[u], 1))

Per-core layouts (core c; g = c%4, m = c//4):
  global row b = 128*t'' + p; this core handles t'' in [16m, 16m+16)
  own w/hist rows: b in [512c, 512c+512), sbuf [p, tl, k], b=512c+128tl+p
  u-shard [4096c, +4096); group table = shards of {g, g+4} (AllGather)
  histogram u = 128*hi + lo, psum [lo, hi]; E'-shard row u_loc = 32p + r
  pair i = 8192j + 128*(32*tl + k) + p, t''loc = 2j + tl; gathered rows
  are group-table fp16, weighted by w*valid, k-reduced by an fp16
  pairwise-add tree (packed 2x DVE), final level to f32.
"""

import os
import numpy as np
from contextlib import ExitStack

import concourse.tile as tile
from concourse import bass, bacc, mybir
from concourse.bass_utils import run_bass_kernel_spmd

dt = mybir.dt
Alu = mybir.AluOpType
Act = mybir.ActivationFunctionType

B, K, U, V, D = 4096, 32, 32768, 100000, 128
NC = 8
BC = B // NC            # 512 output rows per core
USH = U // NC           # 4096 unique ids per core (u-shard)
R = USH // 128          # 32 shard rows per partition
TL = 4                  # own w/hist rows: 4 t''-slices of 128 rows
GM = 2                  # cores per table group
GT = GM * USH           # 8192 rows in the group table
MH = 32 // GM           # 16 t''-slots processed per core
NPC = B // GM * K       # 65536 pairs per core
NCH = 16                # pair chunks
CP = NPC // NCH         # 8192 pairs per chunk
GPC = CP // 128         # 64 gather groups per chunk
TCH = MH // NCH         # 2 t''-slots per chunk

LAST_RESULTS = None     # test harness reads profiling info from here
_PROGRAM = None


def _build_program():
    skips = set(os.environ.get("GCN_SKIP", "").split(","))
    nc = bacc.Bacc("TRN2", target_bir_lowering=False, debug=False, num_devices=NC)

    t_xo = nc.dram_tensor("xo", [128, TL * K], dt.int32, kind="ExternalInput").ap()
    t_valid = nc.dram_tensor("valid", [128, MH * K], dt.float32,
                             kind="ExternalInput").ap()
    t_ids = nc.dram_tensor("ids", [128, R], dt.int32, kind="ExternalInput").ap()
    t_emb = nc.dram_tensor("emb", [V, D], dt.float32, kind="ExternalInput").ap()
    t_iota = nc.dram_tensor("iotaf", [128, 256], dt.float32,
                            kind="ExternalInput").ap()
    t_c127 = nc.dram_tensor("c127", [128, TL * K], dt.int32,
                            kind="ExternalInput").ap()
    t_idxw = nc.dram_tensor("idxw", [128, NPC // 16], dt.int16,
                            kind="ExternalInput").ap()
    t_id128 = nc.dram_tensor("id128", [128, 128], dt.float32,
                             kind="ExternalInput").ap()
    t_out = nc.dram_tensor("out", [BC, D], dt.float32, kind="ExternalOutput").ap()

    # standalone DRAM scratch (offset-0 APs for collectives / indirect reads)
    t_cnt_in = nc.dram_tensor("cnt_in", [256, 128], dt.float32).ap()
    t_cnt_rs = nc.dram_tensor("cnt_rs", [32, 128], dt.float32).ap()
    t_icnf = nc.dram_tensor("icnf", [USH], dt.float32).ap()
    t_ec = nc.dram_tensor("ec", [USH, D], dt.float16).ap()
    t_eg = nc.dram_tensor("eg", [GT, D], dt.float16).ap()
    t_w_in = nc.dram_tensor("w_in", [BC * K], dt.float32).ap()
    t_w_half = nc.dram_tensor("w_half", [4 * BC * K], dt.float32).ap()
    t_po = nc.dram_tensor("po", [B // GM, D], dt.float32).ap()
    t_rso = nc.dram_tensor("rso", [BC, D], dt.float32).ap()

    g_w = [[0, 1, 2, 3], [4, 5, 6, 7]]          # w-AllGather / out-RS groups
    g_e = [[0, 4], [1, 5], [2, 6], [3, 7]]      # table-AllGather groups

    with tile.TileContext(nc) as tc, ExitStack() as ctx:
        sb = ctx.enter_context(tc.tile_pool(name="sb", bufs=1))
        gpool = ctx.enter_context(tc.tile_pool(name="gp", bufs=6))
        tp = ctx.enter_context(tc.tile_pool(name="tp", bufs=2))
        ps = ctx.enter_context(tc.tile_pool(name="ps", bufs=1, space="PSUM"))
        psd = ctx.enter_context(tc.tile_pool(name="psd", bufs=2, space="PSUM"))

        # ---------- loads ----------
        s_ids = sb.tile([128, R], dt.int32)
        nc.sync.dma_start(s_ids[:], t_ids)
        s_xo = sb.tile([128, TL, K], dt.int32)
        nc.sync.dma_start(s_xo[:], t_xo.rearrange("p (t k) -> p t k", t=TL))
        s_iota = sb.tile([128, 256], dt.float32)
        nc.sync.dma_start(s_iota[:], t_iota)
        s_c127 = sb.tile([128, TL, K], dt.int32)
        nc.sync.dma_start(s_c127[:], t_c127.rearrange("p (t k) -> p t k", t=TL))
        s_id128 = sb.tile([128, 128], dt.float32)
        nc.sync.dma_start(s_id128[:], t_id128)
        s_idxw = sb.tile([128, NPC // 16], dt.int16)
        nc.sync.dma_start(s_idxw[:], t_idxw)
        s_valid = sb.tile([128, MH, K], dt.float32)
        nc.sync.dma_start(s_valid[:], t_valid.rearrange("p (t k) -> p t k", t=MH))

        # ---------- E'-shard raw gather (early; Pool engine) ----------
        s_eraw = sb.tile([128, R, D], dt.float32)
        for r in range(1 if "ebuild" in skips else R):
            nc.gpsimd.indirect_dma_start(
                out=s_eraw[:, r, :], out_offset=None, in_=t_emb,
                in_offset=bass.IndirectOffsetOnAxis(ap=s_ids[:, r:r + 1], axis=0))

        # ---------- first-occurrence mask + row weights (own 512 rows) ----------
        s_xf = sb.tile([128, TL, K], dt.float32)
        nc.vector.tensor_copy(s_xf[:], s_xo[:])
        s_dup = sb.tile([128, TL, K], dt.float32)
        nc.vector.memset(s_dup[:], 0.0)
        s_eq = sb.tile([128, TL, K], dt.float32)
        for sh in range(1, (2 if "focc" in skips else K)):
            w = K - sh
            nc.vector.tensor_tensor(
                out=s_eq[:, :, :w], in0=s_xf[:, :, sh:K], in1=s_xf[:, :, 0:w],
                op=Alu.is_equal)
            nc.vector.tensor_tensor(
                out=s_dup[:, :, sh:K], in0=s_dup[:, :, sh:K], in1=s_eq[:, :, :w],
                op=Alu.add)
        s_f = sb.tile([128, TL, K], dt.float32)
        nc.vector.tensor_scalar(
            out=s_f[:], in0=s_dup[:], scalar1=0.0, scalar2=None, op0=Alu.is_equal)
        s_rcnt = sb.tile([128, TL], dt.float32)
        nc.vector.tensor_reduce(
            out=s_rcnt[:], in_=s_f[:], axis=mybir.AxisListType.X, op=Alu.add)
        s_rsq = sb.tile([128, TL], dt.float32)
        nc.scalar.activation(out=s_rsq[:], in_=s_rcnt[:], func=Act.Sqrt)
        s_rn = sb.tile([128, TL], dt.float32)
        nc.vector.reciprocal(out=s_rn[:], in_=s_rsq[:])
        s_s = sb.tile([128, TL, K], dt.float32)
        nc.vector.tensor_tensor(
            out=s_s[:], in0=s_f[:], in1=s_rn[:].to_broadcast([128, TL, K]),
            op=Alu.mult)

        # ---------- histogram of own 512 rows: psum[lo, hi] ----------
        s_lo = sb.tile([128, TL, K], dt.int32)
        nc.vector.tensor_tensor(
            out=s_lo[:], in0=s_xo[:], in1=s_c127[:], op=Alu.bitwise_and)
        s_lof = sb.tile([128, TL, K], dt.float32)
        nc.vector.tensor_copy(s_lof[:], s_lo[:])
        s_hif = sb.tile([128, TL, K], dt.float32)
        nc.vector.tensor_tensor(
            out=s_hif[:], in0=s_xf[:], in1=s_lof[:], op=Alu.subtract)
        s_hifs = sb.tile([128, TL, K], dt.float32)
        nc.vector.tensor_scalar(
            out=s_hifs[:], in0=s_hif[:], scalar1=1.0 / 128.0, scalar2=None,
            op0=Alu.mult)

        p_hist = ps.tile([128, 256], dt.float32, space="PSUM")
        s_iota16 = sb.tile([128, 256], dt.bfloat16)
        nc.vector.tensor_copy(s_iota16[:], s_iota[:])
        ntl = 1 if "hist" in skips else TL
        for tl in range(ntl):
            lo_oh = sb.tile([128, K, 128], dt.bfloat16, tag="looh")
            hi_oh = sb.tile([128, K, 256], dt.bfloat16, tag="hioh")
            for k in range(K):
                nc.vector.tensor_scalar(
                    out=lo_oh[:, k, :], in0=s_iota16[:, 0:128],
                    scalar1=s_lof[:, tl, k:k + 1], scalar2=None,
                    op0=Alu.is_equal)
                nc.vector.tensor_scalar(
                    out=hi_oh[:, k, :], in0=s_iota16[:, 0:256],
                    scalar1=s_hifs[:, tl, k:k + 1], scalar2=s_f[:, tl, k:k + 1],
                    op0=Alu.is_equal, op1=Alu.mult)
            for k in range(K):
                nc.tensor.matmul(
                    p_hist[:], lhsT=lo_oh[:, k, :], rhs=hi_oh[:, k, :],
                    start=(tl == 0 and k == 0),
                    stop=(tl == ntl - 1 and k == K - 1))
        s_hist = sb.tile([128, 256], dt.float32)
        nc.vector.tensor_copy(s_hist[:], p_hist[:])

        # transpose to [hi, lo] u-major layout for the ReduceScatter
        s_tr2 = sb.tile([128, 2, 128], dt.float32)
        for h in range(2):
            p_tr = psd.tile([128, 128], dt.float32, space="PSUM", tag="ptr")
            nc.tensor.transpose(
                out=p_tr[:], in_=s_hist[:, 128 * h:128 * (h + 1)],
                identity=s_id128[:])
            nc.vector.tensor_copy(s_tr2[:, h, :], p_tr[:])
        nc.sync.dma_start(
            t_cnt_in.rearrange("(h p) c -> p h c", h=2), s_tr2[:])
        if "coll" in skips:
            nc.gpsimd.dma_start(t_cnt_rs[:], t_cnt_in[0:32, :])
        else:
            nc.gpsimd.collective_compute(
                "ReduceScatter", Alu.add, replica_groups=[list(range(NC))],
                ins=[t_cnt_in], outs=[t_cnt_rs])

        # ---------- AllGather of row weights within the half-B group ----------
        nc.sync.dma_start(
            t_w_in.rearrange("(p q) -> p q", p=128),
            s_s[:].rearrange("p t k -> p (t k)"))
        if "coll" in skips:
            nc.gpsimd.dma_start(t_w_half[0:BC * K], t_w_in[:])
        else:
            nc.gpsimd.collective_compute(
                "AllGather", Alu.bypass, replica_groups=g_w,
                ins=[t_w_in], outs=[t_w_half])
        s_w = sb.tile([128, MH, K], dt.float32)
        nc.sync.dma_start(
            s_w[:].rearrange("p (c t) k -> p c t k", c=4),
            t_w_half.rearrange("(c p t k) -> p c t k", c=4, p=128, t=TL))
        nc.vector.tensor_tensor(
            out=s_w[:], in0=s_w[:], in1=s_valid[:], op=Alu.mult)
        s_w16 = sb.tile([128, MH, K], dt.float16)
        nc.vector.tensor_copy(s_w16[:], s_w[:])

        # ---------- icn = rsqrt(max(cnt, 1)) for own shard; relayout ----------
        s_csh = sb.tile([32, 128], dt.float32)
        nc.sync.dma_start(s_csh[:], t_cnt_rs)
        s_icsq = sb.tile([32, 128], dt.float32)
        nc.vector.tensor_scalar(
            out=s_icsq[:], in0=s_csh[:], scalar1=1.0, scalar2=None, op0=Alu.max)
        nc.scalar.activation(out=s_icsq[:], in_=s_icsq[:], func=Act.Sqrt)
        s_icn = sb.tile([32, 128], dt.float32)
        nc.vector.reciprocal(out=s_icn[:], in_=s_icsq[:])
        nc.sync.dma_start(t_icnf.rearrange("(h l) -> h l", h=32), s_icn[:])
        s_icnpr = sb.tile([128, R], dt.float32)
        nc.sync.dma_start(s_icnpr[:], t_icnf.rearrange("(p r) -> p r", p=128))

        # ---------- scale shard to fp16, stage, AllGather the group table ----
        s_ec = sb.tile([128, R, D], dt.float16)
        nc.vector.tensor_tensor(
            out=s_ec[:], in0=s_eraw[:],
            in1=s_icnpr[:].to_broadcast([128, R, D]), op=Alu.mult)
        nc.sync.dma_start(t_ec.rearrange("(p r) d -> p r d", p=128), s_ec[:])
        if "coll" in skips:
            nc.gpsimd.dma_start(t_eg[0:USH, :], t_ec[:])
        else:
            nc.gpsimd.collective_compute(
                "AllGather", Alu.bypass, replica_groups=g_e,
                ins=[t_ec], outs=[t_eg])

        # ---------- main gather + masked weighted k-reduction ----------
        s_acc = sb.tile([128, MH, D], dt.float32)
        if "wsum" in skips or "gather" in skips:
            nc.vector.memset(s_acc[:], 0.0)
        for j in range(0 if "gather" in skips else NCH):
            s_g = gpool.tile([128, GPC, D], dt.float16, tag="gch")
            nc.gpsimd.dma_gather(
                out_ap=s_g[:], in_ap=t_eg,
                idxs_ap=s_idxw[:, (CP // 16) * j:(CP // 16) * (j + 1)],
                num_idxs=CP, num_idxs_reg=CP, elem_size=D,
                single_packet=False)
            if "wsum" in skips:
                continue
            gv = s_g[:].rearrange("p (t k) d -> p t k d", t=TCH)
            nc.vector.tensor_tensor(
                out=gv, in0=gv,
                in1=s_w16[:, TCH * j:TCH * (j + 1), :]
                    .to_broadcast([128, TCH, K, D]),
                op=Alu.mult)
            # packed fp16 pairwise-add tree over k (2x DVE), final level to f32
            s_h = tp.tile([128, TCH, K // 2, D], dt.float16, tag="htree")
            nc.vector.tensor_tensor(
                out=s_h[:], in0=gv[:, :, 0:K // 2, :], in1=gv[:, :, K // 2:K, :],
                op=Alu.add)
            m = K // 4
            while m >= 2:
                nc.vector.tensor_tensor(
                    out=s_h[:, :, 0:m, :], in0=s_h[:, :, 0:m, :],
                    in1=s_h[:, :, m:2 * m, :], op=Alu.add)
                m //= 2
            nc.vector.tensor_tensor(
                out=s_acc[:, TCH * j:TCH * (j + 1), :],
                in0=s_h[:, :, 0, :], in1=s_h[:, :, 1, :], op=Alu.add)

        # po row = 128*t''loc + p  (global row b = 2048*(c//4) + po row)
        nc.sync.dma_start(t_po.rearrange("(t p) d -> p t d", p=128), s_acc[:])
        if "coll" in skips:
            nc.gpsimd.dma_start(t_rso[:], t_po[0:BC, :])
        else:
            nc.gpsimd.collective_compute(
                "ReduceScatter", Alu.add, replica_groups=g_w,
                ins=[t_po], outs=[t_rso])
        s_out = sb.tile([128, TL, D], dt.float32)
        nc.sync.dma_start(s_out[:], t_rso.rearrange("(p t) d -> p t d", t=TL))
        nc.sync.dma_start(t_out.rearrange("(p t) d -> p t d", t=TL), s_out[:])

    nc.compile()
    return nc


def _get_program():
    global _PROGRAM
    if _PROGRAM is None:
        _PROGRAM = _build_program()
    return _PROGRAM


def _make_in_maps(neigh_cols, unique_ids, embed_table):
    x = np.ascontiguousarray(np.asarray(neigh_cols, dtype=np.int32))
    uids = np.ascontiguousarray(np.asarray(unique_ids, dtype=np.int32))
    emb = np.ascontiguousarray(np.asarray(embed_table, dtype=np.float32))
    iotaf = np.broadcast_to(np.arange(256, dtype=np.float32), (128, 256)).copy()
    c127 = np.full((128, TL * K), 127, np.int32)
    id128 = np.eye(128, dtype=np.float32)

    # pair order: i = 8192j + 128*(32*tl + k) + p ; t''loc = 2j + tl ;
    # global row b = 2048*(c//4) + 128*t''loc + p
    i = np.arange(NPC)
    j, rem = np.divmod(i, CP)
    g_, p = np.divmod(rem, 128)
    tl, k = np.divmod(g_, K)
    tloc = TCH * j + tl

    pp = np.arange(128)[:, None, None]
    tt = np.arange(MH)[None, :, None]
    kk = np.arange(K)[None, None, :]

    in_maps = []
    for c in range(NC):
        g, m = c % 4, c // 4
        b = 2048 * m + 128 * tloc + p
        ub = x[b, k]
        blk = ub >> 12
        lidx = np.where(
            blk == g, ub - USH * g,
            np.where(blk == g + 4, USH + ub - USH * (g + 4), 0)).astype(np.int16)
        idxw = np.zeros((16, NPC // 16), np.int16)
        idxw[i % 16, i // 16] = lidx
        idxw = np.tile(idxw, (8, 1))

        x_ptk = x[2048 * m + 128 * tt + pp, kk]               # [128, MH, K]
        valid = ((x_ptk >> 12 == g) | (x_ptk >> 12 == g + 4)).astype(np.float32)
        x_own = x[512 * c + 128 * tt[:, 0:TL, :] + pp, kk]    # [128, TL, K]
        ids_c = uids[USH * c:USH * (c + 1)].reshape(128, R)
        in_maps.append({
            "xo": np.ascontiguousarray(x_own.reshape(128, TL * K)),
            "valid": np.ascontiguousarray(valid.reshape(128, MH * K)),
            "ids": ids_c,
            "emb": emb,
            "iotaf": iotaf,
            "c127": c127,
            "idxw": idxw,
            "id128": id128,
        })
    return in_maps


def kernel(neigh_cols, unique_ids, embed_table):
    global LAST_RESULTS
    nc = _get_program()
    in_maps = _make_in_maps(neigh_cols, unique_ids, embed_table)
    trace = bool(int(os.environ.get("GCN_TRACE", "0")))
    res = run_bass_kernel_spmd(nc, in_maps, list(range(NC)), trace=trace)
    LAST_RESULTS = res
    out = np.concatenate([res.results[c]["out"] for c in range(NC)], axis=0)
    return out.astype(np.float32)


def bench_exec(inputs, iters=12):
    """Steady-state wall times (us) of the compiled NEFF via a reusable
    sharded jit with device-resident inputs. Excludes compile; includes
    per-call dispatch overhead of the runtime."""
    import time
    import jax
    from jax.sharding import Mesh, PartitionSpec, NamedSharding
    from jax.experimental.shard_map import shard_map
    from concourse.bass2jax import (_bass_exec_p, partition_id_tensor,
                                    install_neuronx_cc_hook)

    nc = _get_program()
    install_neuronx_cc_hook()
    in_maps = _make_in_maps(**inputs)

    partition_name = (nc.partition_id_tensor.name
                      if nc.partition_id_tensor else None)
    in_names, out_names, out_avals, zero_outs = [], [], [], []
    for alloc in nc.m.functions[0].allocations:
        if not isinstance(alloc, mybir.MemoryLocationSet):
            continue
        name = alloc.memorylocations[0].name
        if alloc.kind == "ExternalInput":
            if name != partition_name:
                in_names.append(name)
        elif alloc.kind == "ExternalOutput":
            out_names.append(name)
            shape = tuple(alloc.tensor_shape)
            npdt = dt.np(alloc.dtype)
            out_avals.append(jax.core.ShapedArray(shape, npdt))
            zero_outs.append(np.zeros(shape, npdt))
    n_params = len(in_names)
    all_names = in_names + out_names + ([partition_name] if partition_name else [])

    def _body(*args):
        operands = list(args)
        if partition_name is not None:
            operands.append(partition_id_tensor())
        return tuple(_bass_exec_p.bind(
            *operands, out_avals=tuple(out_avals), in_names=tuple(all_names),
            out_names=tuple(out_names), lowering_input_output_aliases=(),
            sim_require_finite=True, sim_require_nnan=True, nc=nc))

    devices = jax.devices()[:NC]
    mesh = Mesh(np.asarray(devices), ("core",))
    sharded = jax.jit(
        shard_map(_body, mesh=mesh,
                  in_specs=(PartitionSpec("core"),) * (n_params + len(out_names)),
                  out_specs=(PartitionSpec("core"),) * len(out_names),
                  check_rep=False),
        keep_unused=True)
    sh = NamedSharding(mesh, PartitionSpec("core"))
    concat_in = [jax.device_put(
        np.concatenate([np.asarray(in_maps[c][nm]) for c in range(NC)], axis=0),
        sh) for nm in in_names]
    concat_zero = [jax.device_put(
        np.zeros((NC * z.shape[0], *z.shape[1:]), z.dtype), sh)
        for z in zero_outs]
    out = sharded(*concat_in, *concat_zero)
    jax.block_until_ready(out)
    times = []
    for _ in range(iters):
        t0 = time.perf_counter()
        out = sharded(*concat_in, *concat_zero)
        jax.block_until_ready(out)
        times.append((time.perf_counter() - t0) * 1e6)
    return sorted(times)


def modeled_time_ns():
    """Single-core device-occupancy model of the program (cost-model sim)."""
    from concourse.timeline_sim import TimelineSim
    return TimelineSim(_get_program(), trace=False).simulate()


# revision 25
# speedup vs baseline: 2.6477x; 2.2806x over previous
"""GCN aggregator kernel for 8 Trainium2 NeuronCores (Bass/Tile), v3.

Computes: out = D_r^{-1/2} M D_c^{-1/2} E[unique_ids]  where M is the
[B, U] 0/1 neighbor mask built from neigh_cols (duplicate (row, col)
pairs collapse to 1).

v3 layout ("compute at u-home, 2-core table groups, reduce outputs"):
the v1 bottleneck was a 16.8 MB AllGather of the scaled table E'
(265 us of a 400 us kernel in the collective cost model). Instead:
  - each core builds its 4096-row u-shard of E' = icn * E[ids] locally
    and AllGathers it only within a 2-core group {c%4, c%4+4}
    (2 MB fp16 out, ~67 us), giving the group an 8192-row table;
  - each core processes HALF of B (rows [2048*(c//4), +2048)) x K pairs
    against the group table (out-of-group pairs masked to 0), so the
    descriptor-bound pair gather and the DVE weighted-sum halve vs. an
    all-pairs-per-core scheme;
  - row weights w = f * rsqrt(row_cnt) are computed for 512 own rows
    and AllGathered within [[0..3],[4..7]] (256 KB out);
  - the column-count histogram is ReduceScattered over all 8 cores;
  - partial outputs [2048, D] are ReduceScattered within
    [[0..3],[4..7]], landing each core exactly its [512, D] output.

Sparse decomposition (exact):
  f[b,k]   = 1 if k is the first position in row b with value neigh_cols[b,k]
  row_cnt  = sum_k f[b,k]            (distinct neighbors per row)
  col_cnt  = scatter-add of f by u   (rows containing u; global over B)
  out[b]   = sum_k f[b,k]*rsqrt(row_cnt[b]) * icn[u] * E[unique_ids[u]],
             u = neigh_cols[b,k],  icn[u] = rsqrt(max(col_cnt[u], 1))

Per-core layouts (core c; g = c%4, m = c//4):
  global row b = 128*t'' + p; this core handles t'' in [16m, 16m+16)
  own w/hist rows: b in [512c, 512c+512), sbuf [p, tl, k], b=512c+128tl+p
  u-shard [4096c, +4096); group table = shards of {g, g+4} (AllGather)
  histogram u = 128*hi + lo, psum [lo, hi]; E'-shard row u_loc = 32p + r
  pair i = CP*j + 128*(K*tl + k) + p  (CP pairs per chunk, t''loc =
  TCH*j + tl); gathered rows are group-table fp16, weighted by w*valid,
  k-reduced by an fp16 pairwise-add tree (packed 2x DVE), final to f32.
"""

import os
import numpy as np
from contextlib import ExitStack

import concourse.tile as tile
from concourse import bass, bacc, mybir
from concourse.bass_utils import run_bass_kernel_spmd

dt = mybir.dt
Alu = mybir.AluOpType
Act = mybir.ActivationFunctionType

B, K, U, V, D = 4096, 32, 32768, 100000, 128
NC = 8
BC = B // NC            # 512 output rows per core
USH = U // NC           # 4096 unique ids per core (u-shard)
R = USH // 128          # 32 shard rows per partition
TL = 4                  # own w/hist rows: 4 t''-slices of 128 rows
GM = 2                  # cores per table group
GT = GM * USH           # 8192 rows in the group table
MH = 32 // GM           # 16 t''-slots processed per core
NPC = B // GM * K       # 65536 pairs per core
NCH = 16                # pair chunks
CP = NPC // NCH         # 8192 pairs per chunk
GPC = CP // 128         # 64 gather groups per chunk
TCH = MH // NCH         # 2 t''-slots per chunk

LAST_RESULTS = None     # test harness reads profiling info from here
_PROGRAM = None


def _build_program():
    skips = set(os.environ.get("GCN_SKIP", "").split(","))
    nc = bacc.Bacc("TRN2", target_bir_lowering=False, debug=False, num_devices=NC)

    t_xo = nc.dram_tensor("xo", [128, TL * K], dt.int32, kind="ExternalInput").ap()
    t_valid = nc.dram_tensor("valid", [128, MH * K], dt.float32,
                             kind="ExternalInput").ap()
    t_ids = nc.dram_tensor("ids", [128, R], dt.int32, kind="ExternalInput").ap()
    t_emb = nc.dram_tensor("emb", [V, D], dt.float32, kind="ExternalInput").ap()
    t_iota = nc.dram_tensor("iotaf", [128, 256], dt.float32,
                            kind="ExternalInput").ap()
    t_c127 = nc.dram_tensor("c127", [128, TL * K], dt.int32,
                            kind="ExternalInput").ap()
    t_idxw = nc.dram_tensor("idxw", [128, NPC // 16], dt.int16,
                            kind="ExternalInput").ap()
    t_id128 = nc.dram_tensor("id128", [128, 128], dt.float32,
                             kind="ExternalInput").ap()
    t_out = nc.dram_tensor("out", [BC, D], dt.float32, kind="ExternalOutput").ap()

    # standalone DRAM scratch (offset-0 APs for collectives / indirect reads)
    t_cnt_in = nc.dram_tensor("cnt_in", [256, 128], dt.float32).ap()
    t_cnt_rs = nc.dram_tensor("cnt_rs", [32, 128], dt.float32).ap()
    t_icnf = nc.dram_tensor("icnf", [USH], dt.float32).ap()
    t_ec = nc.dram_tensor("ec", [USH, D], dt.float16).ap()
    t_eg = nc.dram_tensor("eg", [GT, D], dt.float16).ap()
    t_w_in = nc.dram_tensor("w_in", [BC * K], dt.float32).ap()
    t_w_half = nc.dram_tensor("w_half", [4 * BC * K], dt.float32).ap()
    t_po = nc.dram_tensor("po", [B // GM, D], dt.float32).ap()
    t_rso = nc.dram_tensor("rso", [BC, D], dt.float32).ap()

    g_w = [[0, 1, 2, 3], [4, 5, 6, 7]]          # w-AllGather / out-RS groups
    g_e = [[0, 4], [1, 5], [2, 6], [3, 7]]      # table-AllGather groups

    with tile.TileContext(nc) as tc, ExitStack() as ctx:
        sb = ctx.enter_context(tc.tile_pool(name="sb", bufs=1))
        gpool = ctx.enter_context(tc.tile_pool(name="gp", bufs=6))
        tp = ctx.enter_context(tc.tile_pool(name="tp", bufs=2))
        ps = ctx.enter_context(tc.tile_pool(name="ps", bufs=1, space="PSUM"))
        psd = ctx.enter_context(tc.tile_pool(name="psd", bufs=2, space="PSUM"))

        # ---------- loads ----------
        s_ids = sb.tile([128, R], dt.int32)
        nc.sync.dma_start(s_ids[:], t_ids)
        s_xo = sb.tile([128, TL, K], dt.int32)
        nc.sync.dma_start(s_xo[:], t_xo.rearrange("p (t k) -> p t k", t=TL))
        s_iota = sb.tile([128, 256], dt.float32)
        nc.sync.dma_start(s_iota[:], t_iota)
        s_c127 = sb.tile([128, TL, K], dt.int32)
        nc.sync.dma_start(s_c127[:], t_c127.rearrange("p (t k) -> p t k", t=TL))
        s_id128 = sb.tile([128, 128], dt.float32)
        nc.sync.dma_start(s_id128[:], t_id128)
        s_idxw = sb.tile([128, NPC // 16], dt.int16)
        nc.sync.dma_start(s_idxw[:], t_idxw)
        s_valid = sb.tile([128, MH, K], dt.float32)
        nc.sync.dma_start(s_valid[:], t_valid.rearrange("p (t k) -> p t k", t=MH))

        # ---------- E'-shard raw gather (early; Pool engine) ----------
        s_eraw = sb.tile([128, R, D], dt.float32)
        for r in range(1 if "ebuild" in skips else R):
            nc.gpsimd.indirect_dma_start(
                out=s_eraw[:, r, :], out_offset=None, in_=t_emb,
                in_offset=bass.IndirectOffsetOnAxis(ap=s_ids[:, r:r + 1], axis=0))

        # ---------- first-occurrence mask + row weights (own 512 rows) ----------
        s_xf = sb.tile([128, TL, K], dt.float32)
        nc.vector.tensor_copy(s_xf[:], s_xo[:])
        s_dup = sb.tile([128, TL, K], dt.float32)
        nc.vector.memset(s_dup[:], 0.0)
        s_eq = sb.tile([128, TL, K], dt.float32)
        for sh in range(1, (2 if "focc" in skips else K)):
            w = K - sh
            nc.vector.tensor_tensor(
                out=s_eq[:, :, :w], in0=s_xf[:, :, sh:K], in1=s_xf[:, :, 0:w],
                op=Alu.is_equal)
            nc.vector.tensor_tensor(
                out=s_dup[:, :, sh:K], in0=s_dup[:, :, sh:K], in1=s_eq[:, :, :w],
                op=Alu.add)
        s_f = sb.tile([128, TL, K], dt.float32)
        nc.vector.tensor_scalar(
            out=s_f[:], in0=s_dup[:], scalar1=0.0, scalar2=None, op0=Alu.is_equal)
        s_rcnt = sb.tile([128, TL], dt.float32)
        nc.vector.tensor_reduce(
            out=s_rcnt[:], in_=s_f[:], axis=mybir.AxisListType.X, op=Alu.add)
        s_rsq = sb.tile([128, TL], dt.float32)
        nc.scalar.activation(out=s_rsq[:], in_=s_rcnt[:], func=Act.Sqrt)
        s_rn = sb.tile([128, TL], dt.float32)
        nc.vector.reciprocal(out=s_rn[:], in_=s_rsq[:])
        s_s = sb.tile([128, TL, K], dt.float32)
        nc.vector.tensor_tensor(
            out=s_s[:], in0=s_f[:], in1=s_rn[:].to_broadcast([128, TL, K]),
            op=Alu.mult)

        # ---------- histogram of own 512 rows: psum[lo, hi] ----------
        s_lo = sb.tile([128, TL, K], dt.int32)
        nc.vector.tensor_tensor(
            out=s_lo[:], in0=s_xo[:], in1=s_c127[:], op=Alu.bitwise_and)
        s_lof = sb.tile([128, TL, K], dt.float32)
        nc.vector.tensor_copy(s_lof[:], s_lo[:])
        s_hif = sb.tile([128, TL, K], dt.float32)
        nc.vector.tensor_tensor(
            out=s_hif[:], in0=s_xf[:], in1=s_lof[:], op=Alu.subtract)
        s_hifs = sb.tile([128, TL, K], dt.float32)
        nc.vector.tensor_scalar(
            out=s_hifs[:], in0=s_hif[:], scalar1=1.0 / 128.0, scalar2=None,
            op0=Alu.mult)

        p_hist = ps.tile([128, 256], dt.float32, space="PSUM")
        s_iota16 = sb.tile([128, 256], dt.bfloat16)
        nc.vector.tensor_copy(s_iota16[:], s_iota[:])
        ntl = 1 if "hist" in skips else TL
        for tl in range(ntl):
            lo_oh = sb.tile([128, K, 128], dt.bfloat16, tag="looh")
            hi_oh = sb.tile([128, K, 256], dt.bfloat16, tag="hioh")
            for k in range(K):
                nc.vector.tensor_scalar(
                    out=lo_oh[:, k, :], in0=s_iota16[:, 0:128],
                    scalar1=s_lof[:, tl, k:k + 1], scalar2=None,
                    op0=Alu.is_equal)
                nc.vector.tensor_scalar(
                    out=hi_oh[:, k, :], in0=s_iota16[:, 0:256],
                    scalar1=s_hifs[:, tl, k:k + 1], scalar2=s_f[:, tl, k:k + 1],
                    op0=Alu.is_equal, op1=Alu.mult)
            for k in range(K):
                nc.tensor.matmul(
                    p_hist[:], lhsT=lo_oh[:, k, :], rhs=hi_oh[:, k, :],
                    start=(tl == 0 and k == 0),
                    stop=(tl == ntl - 1 and k == K - 1))
        s_hist = sb.tile([128, 256], dt.float32)
        nc.vector.tensor_copy(s_hist[:], p_hist[:])

        # transpose to [hi, lo] u-major layout for the ReduceScatter
        s_tr2 = sb.tile([128, 2, 128], dt.float32)
        for h in range(2):
            p_tr = psd.tile([128, 128], dt.float32, space="PSUM", tag="ptr")
            nc.tensor.transpose(
                out=p_tr[:], in_=s_hist[:, 128 * h:128 * (h + 1)],
                identity=s_id128[:])
            nc.vector.tensor_copy(s_tr2[:, h, :], p_tr[:])
        nc.sync.dma_start(
            t_cnt_in.rearrange("(h p) c -> p h c", h=2), s_tr2[:])
        if "coll" in skips:
            nc.gpsimd.dma_start(t_cnt_rs[:], t_cnt_in[0:32, :])
        else:
            nc.gpsimd.collective_compute(
                "ReduceScatter", Alu.add, replica_groups=[list(range(NC))],
                ins=[t_cnt_in], outs=[t_cnt_rs])

        # ---------- AllGather of row weights within the half-B group ----------
        nc.sync.dma_start(
            t_w_in.rearrange("(p q) -> p q", p=128),
            s_s[:].rearrange("p t k -> p (t k)"))
        if "coll" in skips:
            nc.gpsimd.dma_start(t_w_half[0:BC * K], t_w_in[:])
        else:
            nc.gpsimd.collective_compute(
                "AllGather", Alu.bypass, replica_groups=g_w,
                ins=[t_w_in], outs=[t_w_half])
        s_w = sb.tile([128, MH, K], dt.float32)
        nc.sync.dma_start(
            s_w[:].rearrange("p (c t) k -> p c t k", c=4),
            t_w_half.rearrange("(c p t k) -> p c t k", c=4, p=128, t=TL))
        nc.vector.tensor_tensor(
            out=s_w[:], in0=s_w[:], in1=s_valid[:], op=Alu.mult)
        s_w16 = sb.tile([128, MH, K], dt.float16)
        nc.vector.tensor_copy(s_w16[:], s_w[:])

        # ---------- icn = rsqrt(max(cnt, 1)) for own shard; relayout ----------
        s_csh = sb.tile([32, 128], dt.float32)
        nc.sync.dma_start(s_csh[:], t_cnt_rs)
        s_icsq = sb.tile([32, 128], dt.float32)
        nc.vector.tensor_scalar(
            out=s_icsq[:], in0=s_csh[:], scalar1=1.0, scalar2=None, op0=Alu.max)
        nc.scalar.activation(out=s_icsq[:], in_=s_icsq[:], func=Act.Sqrt)
        s_icn = sb.tile([32, 128], dt.float32)
        nc.vector.reciprocal(out=s_icn[:], in_=s_icsq[:])
        nc.sync.dma_start(t_icnf.rearrange("(h l) -> h l", h=32), s_icn[:])
        s_icnpr = sb.tile([128, R], dt.float32)
        nc.sync.dma_start(s_icnpr[:], t_icnf.rearrange("(p r) -> p r", p=128))

        # ---------- scale shard to fp16, stage, AllGather the group table ----
        s_ec = sb.tile([128, R, D], dt.float16)
        nc.vector.tensor_tensor(
            out=s_ec[:], in0=s_eraw[:],
            in1=s_icnpr[:].to_broadcast([128, R, D]), op=Alu.mult)
        nc.sync.dma_start(t_ec.rearrange("(p r) d -> p r d", p=128), s_ec[:])
        if "coll" in skips:
            nc.gpsimd.dma_start(t_eg[0:USH, :], t_ec[:])
        else:
            nc.gpsimd.collective_compute(
                "AllGather", Alu.bypass, replica_groups=g_e,
                ins=[t_ec], outs=[t_eg])

        # ---------- main gather + masked weighted k-reduction ----------
        s_acc = sb.tile([128, MH, D], dt.float32)
        if "wsum" in skips or "gather" in skips:
            nc.vector.memset(s_acc[:], 0.0)
        for j in range(0 if "gather" in skips else NCH):
            s_g = gpool.tile([128, GPC, D], dt.float16, tag="gch")
            nc.gpsimd.dma_gather(
                out_ap=s_g[:], in_ap=t_eg,
                idxs_ap=s_idxw[:, (CP // 16) * j:(CP // 16) * (j + 1)],
                num_idxs=CP, num_idxs_reg=CP, elem_size=D,
                single_packet=False)
            if "wsum" in skips:
                continue
            gv = s_g[:].rearrange("p (t k) d -> p t k d", t=TCH)
            nc.vector.tensor_tensor(
                out=gv, in0=gv,
                in1=s_w16[:, TCH * j:TCH * (j + 1), :]
                    .to_broadcast([128, TCH, K, D]),
                op=Alu.mult)
            # packed fp16 pairwise-add tree over k (2x DVE), final level to f32
            s_h = tp.tile([128, TCH, K // 2, D], dt.float16, tag="htree")
            nc.vector.tensor_tensor(
                out=s_h[:], in0=gv[:, :, 0:K // 2, :], in1=gv[:, :, K // 2:K, :],
                op=Alu.add)
            m = K // 4
            while m >= 2:
                nc.vector.tensor_tensor(
                    out=s_h[:, :, 0:m, :], in0=s_h[:, :, 0:m, :],
                    in1=s_h[:, :, m:2 * m, :], op=Alu.add)
                m //= 2
            nc.vector.tensor_tensor(
                out=s_acc[:, TCH * j:TCH * (j + 1), :],
                in0=s_h[:, :, 0, :], in1=s_h[:, :, 1, :], op=Alu.add)

        # po row = 128*t''loc + p  (global row b = 2048*(c//4) + po row)
        nc.sync.dma_start(t_po.rearrange("(t p) d -> p t d", p=128), s_acc[:])
        if "coll" in skips:
            nc.gpsimd.dma_start(t_rso[:], t_po[0:BC, :])
        else:
            nc.gpsimd.collective_compute(
                "ReduceScatter", Alu.add, replica_groups=g_w,
                ins=[t_po], outs=[t_rso])
        s_out = sb.tile([128, TL, D], dt.float32)
        nc.sync.dma_start(s_out[:], t_rso.rearrange("(p t) d -> p t d", t=TL))
        nc.sync.dma_start(t_out.rearrange("(p t) d -> p t d", t=TL), s_out[:])

    nc.compile()
    return nc


def _get_program():
    global _PROGRAM
    if _PROGRAM is None:
        _PROGRAM = _build_program()
    return _PROGRAM


def _make_in_maps(neigh_cols, unique_ids, embed_table):
    x = np.ascontiguousarray(np.asarray(neigh_cols, dtype=np.int32))
    uids = np.ascontiguousarray(np.asarray(unique_ids, dtype=np.int32))
    emb = np.ascontiguousarray(np.asarray(embed_table, dtype=np.float32))
    iotaf = np.broadcast_to(np.arange(256, dtype=np.float32), (128, 256)).copy()
    c127 = np.full((128, TL * K), 127, np.int32)
    id128 = np.eye(128, dtype=np.float32)

    # pair order: i = 8192j + 128*(32*tl + k) + p ; t''loc = 2j + tl ;
    # global row b = 2048*(c//4) + 128*t''loc + p
    i = np.arange(NPC)
    j, rem = np.divmod(i, CP)
    g_, p = np.divmod(rem, 128)
    tl, k = np.divmod(g_, K)
    tloc = TCH * j + tl

    pp = np.arange(128)[:, None, None]
    tt = np.arange(MH)[None, :, None]
    kk = np.arange(K)[None, None, :]

    in_maps = []
    for c in range(NC):
        g, m = c % 4, c // 4
        b = 2048 * m + 128 * tloc + p
        ub = x[b, k]
        blk = ub >> 12
        lidx = np.where(
            blk == g, ub - USH * g,
            np.where(blk == g + 4, USH + ub - USH * (g + 4), 0)).astype(np.int16)
        idxw = np.zeros((16, NPC // 16), np.int16)
        idxw[i % 16, i // 16] = lidx
        idxw = np.tile(idxw, (8, 1))

        x_ptk = x[2048 * m + 128 * tt + pp, kk]               # [128, MH, K]
        valid = ((x_ptk >> 12 == g) | (x_ptk >> 12 == g + 4)).astype(np.float32)
        x_own = x[512 * c + 128 * tt[:, 0:TL, :] + pp, kk]    # [128, TL, K]
        ids_c = uids[USH * c:USH * (c + 1)].reshape(128, R)
        in_maps.append({
            "xo": np.ascontiguousarray(x_own.reshape(128, TL * K)),
            "valid": np.ascontiguousarray(valid.reshape(128, MH * K)),
            "ids": ids_c,
            "emb": emb,
            "iotaf": iotaf,
            "c127": c127,
            "idxw": idxw,
            "id128": id128,
        })
    return in_maps


def kernel(neigh_cols, unique_ids, embed_table):
    global LAST_RESULTS
    nc = _get_program()
    in_maps = _make_in_maps(neigh_cols, unique_ids, embed_table)
    trace = bool(int(os.environ.get("GCN_TRACE", "0")))
    res = run_bass_kernel_spmd(nc, in_maps, list(range(NC)), trace=trace)
    LAST_RESULTS = res
    out = np.concatenate([res.results[c]["out"] for c in range(NC)], axis=0)
    return out.astype(np.float32)


def bench_exec(inputs, iters=12):
    """Steady-state wall times (us) of the compiled NEFF via a reusable
    sharded jit with device-resident inputs. Excludes compile; includes
    per-call dispatch overhead of the runtime."""
    import time
    import jax
    from jax.sharding import Mesh, PartitionSpec, NamedSharding
    from jax.experimental.shard_map import shard_map
    from concourse.bass2jax import (_bass_exec_p, partition_id_tensor,
                                    install_neuronx_cc_hook)

    nc = _get_program()
    install_neuronx_cc_hook()
    in_maps = _make_in_maps(**inputs)

    partition_name = (nc.partition_id_tensor.name
                      if nc.partition_id_tensor else None)
    in_names, out_names, out_avals, zero_outs = [], [], [], []
    for alloc in nc.m.functions[0].allocations:
        if not isinstance(alloc, mybir.MemoryLocationSet):
            continue
        name = alloc.memorylocations[0].name
        if alloc.kind == "ExternalInput":
            if name != partition_name:
                in_names.append(name)
        elif alloc.kind == "ExternalOutput":
            out_names.append(name)
            shape = tuple(alloc.tensor_shape)
            npdt = dt.np(alloc.dtype)
            out_avals.append(jax.core.ShapedArray(shape, npdt))
            zero_outs.append(np.zeros(shape, npdt))
    n_params = len(in_names)
    all_names = in_names + out_names + ([partition_name] if partition_name else [])

    def _body(*args):
        operands = list(args)
        if partition_name is not None:
            operands.append(partition_id_tensor())
        return tuple(_bass_exec_p.bind(
            *operands, out_avals=tuple(out_avals), in_names=tuple(all_names),
            out_names=tuple(out_names), lowering_input_output_aliases=(),
            sim_require_finite=True, sim_require_nnan=True, nc=nc))

    devices = jax.devices()[:NC]
    mesh = Mesh(np.asarray(devices), ("core",))
    sharded = jax.jit(
        shard_map(_body, mesh=mesh,
                  in_specs=(PartitionSpec("core"),) * (n_params + len(out_names)),
                  out_specs=(PartitionSpec("core"),) * len(out_names),
                  check_rep=False),
        keep_unused=True)
    sh = NamedSharding(mesh, PartitionSpec("core"))
    concat_in = [jax.device_put(
        np.concatenate([np.asarray(in_maps[c][nm]) for c in range(NC)], axis=0),
        sh) for nm in in_names]
    concat_zero = [jax.device_put(
        np.zeros((NC * z.shape[0], *z.shape[1:]), z.dtype), sh)
        for z in zero_outs]
    out = sharded(*concat_in, *concat_zero)
    jax.block_until_ready(out)
    times = []
    for _ in range(iters):
        t0 = time.perf_counter()
        out = sharded(*concat_in, *concat_zero)
        jax.block_until_ready(out)
        times.append((time.perf_counter() - t0) * 1e6)
    return sorted(times)


def modeled_time_ns():
    """Single-core device-occupancy model of the program (cost-model sim)."""
    from concourse.timeline_sim import TimelineSim
    return TimelineSim(_get_program(), trace=False).simulate()
